# revision 51
# baseline (speedup 1.0000x reference)
"""Trainium2 Bass kernel for nn_DiagonalTraining (anti-diagonal per-diag Linear).

out[b, r, c] = sum_{r'} W[d, r - r0(d), r' - r0(d)] * x[b, r', d - r'] + bias,
with d = r + c, over the valid range of r' for diagonal d.

Strategy: shard the 511 independent diagonals across 8 cores (expert-style),
all data in bf16.
  - long diagonals (n > 128): sorted by n descending, assigned round-robin so
    slot j has ~equal n on every core (SPMD shares one program). Slot matmul
    shapes use the slot max N_j: PSUM[128b, N_j] accumulated over K-chunks
    (128, K2_j = N_j - 128); W shipped as [K, N_j] exact-width columns.
    Chunk2 K-partitions are vertically packed: slots with small K2 "ride" in
    the dead partition rows of a carrier slot's chunk2 block (rider matmuls
    use tile_position=(64|96, 0)), eliminating most zero-padding traffic.
  - short diagonals (n <= 128): pair-packed into bins of K=128 (block-diag
    W), one matmul [K=128] x [N=128] per bin, 17 bins/core.
Stationary operand = gathered diagonal data xd^T [K, batch=128]; moving
operand = per-diagonal weights [K, N]. Jobs are paired two-per-PSUM-bank;
PSUM->SBUF bf16 casts alternate between Vector and Scalar engines; input
DMAs ride the sync-engine HWDGE ring while output DMAs use the scalar ring.
Host scatters the packed outputs back to the grid and adds bias.
"""

import sys

sys.path.insert(0, "/opt/trn_rl_repo")

import numpy as np

B, S = 128, 256
D = 2 * S - 1  # 511
NCORES = 8
NSB = 17  # short-diagonal bins per core
NLJ = 32  # long-diagonal slots per core

TRACE = False  # test.py sets True to pull exec_time_ns from the NTFF profile
last_results = None

# chunk2 vertical packing: rider slot -> (carrier slot, partition base)
CARRIER_OF = {}
for _i in range(8):
    CARRIER_OF[24 + _i] = (8 + _i, 96)
for _i in range(4):
    CARRIER_OF[17 + 2 * _i] = (16 + 2 * _i, 64)

# input-DMA groups over long slots (fine-grained so the PE starts early and
# rides right behind the DMA stream), then groups for the short bins
import os as _os

_LG = _os.environ.get("KCFG_LG", "coarse")
if _LG == "fine":
    LGROUPS = [(0, 2), (2, 5), (5, 11), (11, 22), (22, 32)]
elif _LG == "medium":
    LGROUPS = [(0, 6), (6, 11), (11, 22), (22, 32)]
else:
    LGROUPS = [(0, 11), (11, 22), (22, 32)]
SGROUPS = [(0, 14), (14, 17)]
SHORTS_FIRST = _os.environ.get("KCFG_SF", "0") == "1"
WARMUP_N = int(_os.environ.get("KCFG_WARMN", "64"))
WARMUP_MMS = int(
    _os.environ.get("KCFG_WARMUP", "25" if SHORTS_FIRST else "75")
)
OUT_GPSIMD = _os.environ.get("KCFG_OUTGP", "0") == "1"
TINY_TAIL = _os.environ.get("KCFG_TT", "0") == "1"
HOIST = _os.environ.get("KCFG_HOIST", "0") == "1"
OG_EARLY = _os.environ.get("KCFG_OGE", "0") == "1"
OUT_SYNC = _os.environ.get("KCFG_OUTSYNC", "1") == "1"


def _geom(d):
    r0 = max(0, d - S + 1)
    n = d + 1 if d < S else 2 * S - 1 - d
    return r0, n


def _slot_geom():
    """Long diags sorted by n desc, round-robin to cores; slot-wise max n."""
    longs = [(d, _geom(d)[1]) for d in range(128, 383)]
    longs.sort(key=lambda t: (-t[1], t[0]))
    per_core = [[None] * NLJ for _ in range(NCORES)]
    slot_n = [0] * NLJ
    for i, (d, n) in enumerate(longs):
        c, j = i % NCORES, i // NCORES
        per_core[c][j] = d
        slot_n[j] = max(slot_n[j], n)
    return per_core, slot_n


def _layout(slot_n):
    """Shared column layout: xd blocks, W col offsets, group extents."""
    xd_blocks = []  # (slot, chunk) in slot order
    for j in range(NLJ):
        xd_blocks.append((j, 0))
        if j not in CARRIER_OF:
            xd_blocks.append((j, 1))
    xd_pos = {bc: i for i, bc in enumerate(xd_blocks)}
    lw = [slot_n[j] if j in CARRIER_OF else 2 * slot_n[j] for j in range(NLJ)]
    lwoff = np.concatenate([[0], np.cumsum(lw)]).astype(np.int64)
    yoff = np.concatenate([[0], np.cumsum(slot_n)]).astype(np.int64)
    # per group: (slot range, xd block index range, group col count)
    groups = []
    for (j0, j1) in LGROUPS:
        b0 = xd_pos[(j0, 0)]
        b1 = xd_pos[(j1, 0)] if j1 < NLJ else len(xd_blocks)
        nx = (b1 - b0) * 128
        gc = nx + int(lwoff[j1] - lwoff[j0])
        groups.append((j0, j1, b0, b1, nx, gc))
    return xd_blocks, xd_pos, lwoff, yoff, groups


def _job_tables():
    """Static per-core packing tables (indices + masks + scatter targets)."""
    # ---- short bins: 129 real bins + 7 dummies = 136 = 8 * 17
    sbins = []
    for kk in range(1, 64):
        sbins.append([kk - 1, 127 - kk])
        sbins.append([511 - kk, 383 + kk])
    sbins.append([63, 447])
    sbins.append([127])
    sbins.append([383])
    sbins += [[] for _ in range(136 - len(sbins))]

    long_per_core, slot_n = _slot_geom()
    xd_blocks, xd_pos, lwoff, yoff, groups = _layout(slot_n)
    NXB = len(xd_blocks)
    LWC = int(lwoff[-1])
    LYC = int(yoff[-1])

    cores = []
    for c in range(NCORES):
        my_s = sbins[c::NCORES]
        xds_i = np.zeros((NSB, 128), np.int64)
        xds_m = np.zeros((NSB, 128), np.float32)
        ws_i = np.zeros((NSB, 128, 128), np.int64)
        ws_m = np.zeros((NSB, 128, 128), np.float32)
        tgt_s = np.full((NSB, 128), -1, np.int64)
        for j, bin_ds in enumerate(my_s):
            off = 0
            for d in bin_ds:
                r0, n = _geom(d)
                i = np.arange(n)
                r = r0 + i
                col = d - r
                xds_i[j, off : off + n] = r * S + col
                xds_m[j, off : off + n] = 1.0
                # W[d, m, k] at [k, m] (k = contraction pos, m = output pos)
                ws_i[j, off : off + n, off : off + n] = (
                    d * S * S + i[None, :] * S + i[:, None]
                )
                ws_m[j, off : off + n, off : off + n] = 1.0
                tgt_s[j, off : off + n] = r * S + col
                off += n

        xdl_i = np.zeros((NXB, 128), np.int64)
        xdl_m = np.zeros((NXB, 128), np.float32)
        wl_i = np.zeros((128, LWC), np.int64)
        wl_m = np.zeros((128, LWC), np.float32)
        tgt_l = np.full(LYC, -1, np.int64)

        for j in range(NLJ):
            d = long_per_core[c][j]
            if d is None:
                continue
            r0, n = _geom(d)
            N = slot_n[j]
            m = np.arange(N)
            mv = m < n
            # chunk1: xd block (j, 0), W cols [lwoff[j], +N)
            blk = xd_pos[(j, 0)]
            k = np.arange(128)
            v = k < n
            r = r0 + np.minimum(k, n - 1)
            xdl_i[blk] = (r * S + (d - r)) * v
            xdl_m[blk] = v.astype(np.float32)
            o = int(lwoff[j])
            wm = mv[None, :] & v[:, None]
            wl_i[:, o : o + N] = (
                d * S * S
                + np.minimum(m, n - 1)[None, :] * S
                + np.minimum(k, n - 1)[:, None]
            ) * wm
            wl_m[:, o : o + N] = wm.astype(np.float32)
            # chunk2: either own block or rider rows in carrier's block
            if j in CARRIER_OF:
                cj, base = CARRIER_OF[j]
                blk2 = xd_pos[(cj, 1)]
                rows = np.arange(base, 128)
                k2 = 128 + (rows - base)
                o2 = int(lwoff[cj]) + slot_n[cj]
                N2 = N  # rider W cols: first N of carrier chunk2 block
            else:
                blk2 = xd_pos[(j, 1)]
                rows = np.arange(0, 128)
                k2 = 128 + rows
                o2 = o + N
                N2 = N
            v2 = k2 < n
            r2 = r0 + np.minimum(k2, n - 1)
            xdl_i[blk2, rows] = (r2 * S + (d - r2)) * v2
            xdl_m[blk2, rows] = v2.astype(np.float32)
            wm2 = mv[None, :N2] & v2[:, None]
            wl_i[np.ix_(rows, np.arange(o2, o2 + N2))] = (
                d * S * S
                + np.minimum(m[:N2], n - 1)[None, :] * S
                + np.minimum(k2, n - 1)[:, None]
            ) * wm2
            wl_m[np.ix_(rows, np.arange(o2, o2 + N2))] = wm2.astype(np.float32)
            mr = r0 + m[:n]
            tgt_l[int(yoff[j]) : int(yoff[j]) + n] = mr * S + (d - mr)
        cores.append(
            dict(
                xds_i=xds_i, xds_m=xds_m, ws_i=ws_i, ws_m=ws_m, tgt_s=tgt_s,
                xdl_i=xdl_i, xdl_m=xdl_m, wl_i=wl_i, wl_m=wl_m, tgt_l=tgt_l,
            )
        )
    # bias gather: out_flat[p] += b[d, r - r0(d)] for p = r*S + c, d = r + c
    rr, cc = np.divmod(np.arange(S * S), S)
    dd = rr + cc
    r0v = np.maximum(0, dd - S + 1)
    bidx = dd * S + (rr - r0v)
    return cores, bidx, slot_n, lwoff, yoff, groups, xd_pos


_TABLES = None
_PROG = None


def _tables():
    global _TABLES
    if _TABLES is None:
        _TABLES = _job_tables()
    return _TABLES


def _build_program(slot_n, lwoff, yoff, groups, xd_pos):
    import concourse.bass as bass
    import concourse.mybir as mybir

    f32 = mybir.dt.float32
    bf16 = mybir.dt.bfloat16
    LYC = int(yoff[-1])
    SYC = NSB * 128
    SG = SGROUPS  # short-bin input groups
    # shorts stream (and compute) first: their dense N=128 matmuls double as
    # the HAM warmup while the long groups are still in flight
    sg_cols = [(s1 - s0) * 256 for s0, s1 in SG]
    if SHORTS_FIRST:
        gcols = sg_cols + [g[5] for g in groups]
        l_goff, s_goff = len(SG), 0
    else:
        gcols = [g[5] for g in groups] + sg_cols
        l_goff, s_goff = 0, len(groups)
    NG = len(gcols)
    group_of_slot = {}
    for g, (j0, j1, b0, b1, nx, gc) in enumerate(groups):
        for j in range(j0, j1):
            group_of_slot[j] = l_goff + g

    nc = bass.Bass(enable_partition_id=False)
    bin_t = nc.dram_tensor("bin", [128, sum(gcols)], bf16, kind="ExternalInput")
    yout = nc.dram_tensor("yout", [128, LYC + SYC], bf16, kind="ExternalOutput")

    NPS = 8  # psum banks; one pair of jobs per bank

    BT = [
        nc.alloc_sbuf_tensor(f"bt{g}", [128, gc], bf16).ap()
        for g, gc in enumerate(gcols)
    ]
    YB = nc.alloc_sbuf_tensor("YB", [128, LYC + SYC], bf16).ap()
    PS = [
        nc.alloc_psum_tensor(f"ps{i}", [128, 512], f32).ap() for i in range(NPS)
    ]

    lgroup_of_slot = {}
    for g, (j0, j1, *_r) in enumerate(groups):
        for j in range(j0, j1):
            lgroup_of_slot[j] = g

    def slot_aps(j):
        """(bt, xd1_off, w1_off, bt2, rows_base, xd2_off, w2_off) for slot j."""
        g = lgroup_of_slot[j]
        j0, j1, b0, b1, nx, gc = groups[g]
        xo1 = (xd_pos[(j, 0)] - b0) * 128
        wo1 = nx + int(lwoff[j] - lwoff[j0])
        if j in CARRIER_OF:
            cj, base = CARRIER_OF[j]
            g2 = lgroup_of_slot[cj]
            c0, c1, cb0, cb1, cnx, cgc = groups[g2]
            xo2 = (xd_pos[(cj, 1)] - cb0) * 128
            wo2 = cnx + int(lwoff[cj] - lwoff[c0]) + slot_n[cj]
            return BT[l_goff + g], xo1, wo1, BT[l_goff + g2], base, xo2, wo2
        xo2 = (xd_pos[(j, 1)] - b0) * 128
        wo2 = wo1 + slot_n[j]
        return BT[l_goff + g], xo1, wo1, BT[l_goff + g], 0, xo2, wo2

    # unified job list: (group, kind, j)
    long_jobs = []
    for g, (j0, j1, *_rest) in enumerate(groups):
        for j in range(j0, j1):
            long_jobs.append((l_goff + g, "L", j))
    short_jobs = []
    for gi, (s0, s1) in enumerate(SG):
        for j in range(s0, s1):
            short_jobs.append((s_goff + gi, "S", j))
    jobs = short_jobs + long_jobs if SHORTS_FIRST else long_jobs + short_jobs
    njobs = len(jobs)

    def job_meta(ji):
        g, kind, j = jobs[ji]
        if kind == "L":
            return int(yoff[j]), slot_n[j]
        return LYC + j * 128, 128

    # copy batches: consecutive jobs sharing one PSUM bank (<= 512 f32 cols).
    # One PSUM->SBUF cast per batch, split between Vector and Scalar engines.
    if TINY_TAIL:
        short_szs = (4, 4, 4, 2, 2, 1)  # last bin alone: small final cast/DMA
    else:
        short_szs = (4, 4, 4, 2, 3)  # bins 0-3, 4-7, 8-11, 12-13, 14-16
    batches = []  # (first_job, n_jobs)
    ji = 0
    if SHORTS_FIRST:
        for sz in short_szs:
            batches.append((ji, sz))
            ji += sz
    while ji < njobs - (0 if SHORTS_FIRST else NSB):
        batches.append((ji, 2))
        ji += 2
    if not SHORTS_FIRST:
        for sz in short_szs:
            batches.append((ji, sz))
            ji += sz
    assert ji == njobs
    nbatch = len(batches)
    bwidth = [
        sum(job_meta(j0 + k)[1] for k in range(njn)) for j0, njn in batches
    ]
    assert max(bwidth) <= 512
    batch_of_job = {}
    job_po = {}
    for bi, (j0, njn) in enumerate(batches):
        o = 0
        for k in range(njn):
            batch_of_job[j0 + k] = bi
            job_po[j0 + k] = o
            o += job_meta(j0 + k)[1]

    # greedy engine assignment (vector ~245 G elem/s, scalar ~153 G elem/s;
    # scalar also pays ~650 ns per output-DMA issue)
    if SHORTS_FIRST:
        # late-merged outputs: no out-DMA during the input stream (HBM
        # write contention slows the critical input path)
        og_bounds = [13, 18, nbatch]
    elif TINY_TAIL:
        og_bounds = [6, 11, 16, 19, nbatch - 1, nbatch]
    elif OG_EARLY:
        # drain the output ring during the input stream; only the small
        # final group remains on the exit tail
        og_bounds = [3, 6, 9, 12, 16, 19, nbatch]
    else:
        og_bounds = [6, 11, 16, 19, nbatch]  # og i covers batches [prev, bound)
    eng_of = []
    tv = ts = 0.0
    ogi = 0
    for bi in range(nbatch):
        cv = bwidth[bi] * 1.00 + 350.0
        cs = bwidth[bi] * 1.10 + 350.0
        if tv + cv <= ts + cs:
            eng_of.append("V")
            tv += cv
        else:
            eng_of.append("S")
            ts += cs
        if ogi < len(og_bounds) and bi + 1 == og_bounds[ogi]:
            ts += 650.0
            ogi += 1

    def done_counts(b_end):
        """(n_vector, n_scalar) casts among batches [0, b_end)."""
        nv = sum(1 for e in eng_of[:b_end] if e == "V")
        return nv, b_end - nv

    # each og = (bound, [(c0, c1), ...]): contiguous column runs over the
    # jobs of batches [prev, bound) — two runs when a group straddles the
    # shorts/longs region boundary
    og = []
    prev = 0
    for bnd in og_bounds:
        runs = []
        for bi in range(prev, bnd):
            j0, njn = batches[bi]
            for k in range(njn):
                c0, w = job_meta(j0 + k)
                if runs and runs[-1][1] == c0:
                    runs[-1][1] = c0 + w
                else:
                    runs.append([c0, c0 + w])
        og.append((bnd, [tuple(r) for r in runs]))
        prev = bnd
    n_out_dmas = sum(len(r) for _, r in og)

    DIN = [nc.alloc_semaphore(f"din{i}") for i in range(NG)]
    P = nc.alloc_semaphore("P")  # PE job completions
    CV = nc.alloc_semaphore("CV")  # vector cast completions
    CS = nc.alloc_semaphore("CS")  # scalar cast completions
    DO = nc.alloc_semaphore("DO")  # output DMA completions (x16)

    with nc.Block(no_gpsimd_drain=True) as block:

        @block.sync
        def _(sync):
            goff = 0
            for g, gc in enumerate(gcols):
                sync.dma_start(
                    out=BT[g][:], in_=bin_t[:, goff : goff + gc]
                ).then_inc(DIN[g], 16)
                goff += gc
            if OUT_SYNC:
                for thr, runs in og:
                    nv, ns = done_counts(thr)
                    sync.wait_ge(CV, nv)
                    sync.wait_ge(CS, ns)
                    for c0, c1 in runs:
                        sync.dma_start(
                            out=yout[:, c0:c1], in_=YB[:, c0:c1]
                        ).then_inc(DO, 16)
            sync.wait_ge(DO, 16 * n_out_dmas)

        @block.tensor
        def _(tensor):
            # optional HAM warmup: dummy matmuls while the first input DMA is
            # in flight, so the PE clock-gate opens before real work starts.
            # Results land in PS[0], cleared by the first real batch.
            for _ in range(WARMUP_MMS):
                nc.tensor.matmul(
                    PS[0][0:64, 0:WARMUP_N],
                    BT[0][0:64, 0:64],
                    BT[0][0:64, 64 : 64 + WARMUP_N],
                    start=True,
                    stop=True,
                )
            cur_g = -1
            for ji, (g, kind, j) in enumerate(jobs):
                if g > cur_g:
                    tensor.wait_ge(DIN[g], 16)
                    cur_g = g
                bi = batch_of_job[ji]
                if HOIST and kind == "S" and not SHORTS_FIRST:
                    # one combined recycle-wait before the first short MM so
                    # the 17-MM tail stream runs unpunctuated
                    if ji == NLJ:
                        q = nbatch - 1 - NPS
                        nv, ns = done_counts(q + 1)
                        tensor.wait_ge(CV, nv)
                        tensor.wait_ge(CS, ns)
                elif ji == batches[bi][0] and bi >= NPS:
                    q = bi - NPS
                    sem = CV if eng_of[q] == "V" else CS
                    tensor.wait_ge(sem, done_counts(q + 1)[0 if eng_of[q] == "V" else 1])
                ps = PS[bi % NPS]
                po = job_po[ji]
                if kind == "L":
                    N = slot_n[j]
                    K2 = N - 128
                    bt, xo1, wo1, bt2, base, xo2, wo2 = slot_aps(j)
                    nc.tensor.matmul(
                        ps[:, po : po + N],
                        bt[:, xo1 : xo1 + 128],
                        bt[:, wo1 : wo1 + N],
                        start=True,
                        stop=False,
                    )
                    mm = nc.tensor.matmul(
                        ps[:, po : po + N],
                        bt2[base : base + K2, xo2 : xo2 + 128],
                        bt2[base : base + K2, wo2 : wo2 + N],
                        start=False,
                        stop=True,
                        tile_position=(base, 0),
                    )
                else:
                    s0, s1 = SG[g - s_goff]
                    bt = BT[g]
                    xo = (j - s0) * 128
                    wo = (s1 - s0) * 128 + (j - s0) * 128
                    mm = nc.tensor.matmul(
                        ps[:, po : po + 128],
                        bt[:, xo : xo + 128],
                        bt[:, wo : wo + 128],
                        start=True,
                        stop=True,
                    )
                mm.then_inc(P, 1)

        @block.vector
        def _(vector):
            for bi in range(nbatch):
                if eng_of[bi] != "V":
                    continue
                j0, njn = batches[bi]
                vector.wait_ge(P, j0 + njn)
                ya = job_meta(j0)[0]
                cp = nc.vector.tensor_copy(
                    YB[:, ya : ya + bwidth[bi]], PS[bi % NPS][:, 0 : bwidth[bi]]
                )
                cp.then_inc(CV, 1)

        if OUT_GPSIMD:

            @block.scalar
            def _(scalar):
                for bi in range(nbatch):
                    if eng_of[bi] != "S":
                        continue
                    j0, njn = batches[bi]
                    scalar.wait_ge(P, j0 + njn)
                    ya = job_meta(j0)[0]
                    cp = nc.scalar.copy(
                        YB[:, ya : ya + bwidth[bi]],
                        PS[bi % NPS][:, 0 : bwidth[bi]],
                    )
                    cp.then_inc(CS, 1)

            @block.gpsimd
            def _(gpsimd):
                for thr, runs in og:
                    nv, ns = done_counts(thr)
                    gpsimd.wait_ge(CV, nv)
                    gpsimd.wait_ge(CS, ns)
                    for c0, c1 in runs:
                        gpsimd.dma_start(
                            out=yout[:, c0:c1], in_=YB[:, c0:c1]
                        ).then_inc(DO, 16)

        else:

            @block.scalar
            def _(scalar):
                ogi = len(og) if OUT_SYNC else 0
                for bi in range(nbatch):
                    if eng_of[bi] == "S":
                        j0, njn = batches[bi]
                        scalar.wait_ge(P, j0 + njn)
                        ya = job_meta(j0)[0]
                        cp = nc.scalar.copy(
                            YB[:, ya : ya + bwidth[bi]],
                            PS[bi % NPS][:, 0 : bwidth[bi]],
                        )
                        cp.then_inc(CS, 1)
                    while ogi < len(og) and og[ogi][0] <= bi + 1:
                        thr, runs = og[ogi]
                        nv, ns = done_counts(thr)
                        scalar.wait_ge(CV, nv)
                        scalar.wait_ge(CS, ns)
                        for c0, c1 in runs:
                            scalar.dma_start(
                                out=yout[:, c0:c1], in_=YB[:, c0:c1]
                            ).then_inc(DO, 16)
                        ogi += 1
                while ogi < len(og):
                    thr, runs = og[ogi]
                    nv, ns = done_counts(thr)
                    scalar.wait_ge(CV, nv)
                    scalar.wait_ge(CS, ns)
                    for c0, c1 in runs:
                        scalar.dma_start(
                            out=yout[:, c0:c1], in_=YB[:, c0:c1]
                        ).then_inc(DO, 16)
                    ogi += 1

    return nc


def _get_program():
    global _PROG
    if _PROG is None:
        _, _, slot_n, lwoff, yoff, groups, xd_pos = _tables()
        _PROG = _build_program(slot_n, lwoff, yoff, groups, xd_pos)
    return _PROG


def _pack_core(t, x_flat, W_flat, lwoff, groups, np_dt):
    # long xd: [B, NXB, 128] -> [128k, NXB, B] col blocks
    xdl = x_flat[:, t["xdl_i"]] * t["xdl_m"]
    NXB = t["xdl_i"].shape[0]
    XDL = xdl.transpose(2, 1, 0).reshape(128, NXB * 128)
    WL = W_flat[t["wl_i"]] * t["wl_m"]  # [128, LWC]
    # short xd: [B, NSB, 128] -> [128k, NSB, B]
    xds = x_flat[:, t["xds_i"]] * t["xds_m"]
    XDS = xds.transpose(2, 1, 0).reshape(128, NSB * 128)
    ws = W_flat[t["ws_i"]] * t["ws_m"]  # [NSB, 128k, 128m]
    WS = ws.transpose(1, 0, 2).reshape(128, NSB * 128)
    lparts = []
    for (j0, j1, b0, b1, nx, gc) in groups:
        lparts.append(XDL[:, b0 * 128 : b1 * 128])
        lparts.append(WL[:, int(lwoff[j0]) : int(lwoff[j1])])
    sparts = []
    for s0, s1 in SGROUPS:
        sparts.append(XDS[:, s0 * 128 : s1 * 128])
        sparts.append(WS[:, s0 * 128 : s1 * 128])
    parts = sparts + lparts if SHORTS_FIRST else lparts + sparts
    bin_arr = np.concatenate(parts, axis=1).astype(np_dt)
    return {"bin": np.ascontiguousarray(bin_arr)}


def kernel(x, W, b):
    import ml_dtypes
    from concourse.bass_utils import run_bass_kernel_spmd

    x = np.asarray(x, np.float32)
    W = np.asarray(W, np.float32)
    b = np.asarray(b, np.float32)
    cores, bidx, slot_n, lwoff, yoff, groups, xd_pos = _tables()
    np_dt = ml_dtypes.bfloat16
    x_flat = x.reshape(B, S * S)
    W_flat = W.reshape(-1)
    in_maps = [
        _pack_core(t, x_flat, W_flat, lwoff, groups, np_dt) for t in cores
    ]
    nc = _get_program()
    res = run_bass_kernel_spmd(
        nc, in_maps, core_ids=list(range(NCORES)), trace=TRACE
    )
    global last_results
    last_results = res
    LYC = int(yoff[-1])
    out_flat = np.zeros((B, S * S), np.float32)
    for c, t in enumerate(cores):
        yv = np.asarray(res.results[c]["yout"]).astype(np.float32)
        yv = yv.reshape(B, -1)
        fl = t["tgt_l"]
        vl = fl >= 0
        out_flat[:, fl[vl]] = yv[:, :LYC][:, vl]
        fs = t["tgt_s"].reshape(-1)
        vs = fs >= 0
        out_flat[:, fs[vs]] = yv[:, LYC:][:, vs]
    out_flat += b.reshape(-1)[bidx][None, :]
    return out_flat.reshape(B, S, S)
